# revision 4
# baseline (speedup 1.0000x reference)
"""Cross-attention Trainium2 Bass kernel.

Reference computation (per batch b):
  q = x @ Wq;  k = ctx @ Wk;  v = ctx @ Wv        (16 heads x 64 dim)
  sim = q k^T / 8;  attn = softmax(sim);  out = (attn v) @ Wo + bo

Sharding: 8 cores = 4 batches x 2 head-groups (8 heads each).
Each core computes a partial output [2048, 1024] (its 8 heads' contribution
through Wo); the host sums the two partials per batch and adds the bias.

Per-core data layout (host-prepared, all fp32 bits run as float32r on PE):
  xT   [1024, 2048]  = x[b].T               (contraction dim on partitions)
  ctxT [ 768, 2048]  = context[b].T
  wq   [1024,  512]  = Wq[:, g*512:+512] * 0.125   (attn scale folded in)
  wk   [ 768,  512]  = Wk[:, g*512:+512]
  wv   [ 768,  512]  = Wv[:, g*512:+512]
  wo   [ 512, 1024]  = Wo[g*512:+512, :]

Device pipeline:
  A: Q^T [512, 2048] = wq^T @ xT        (psum; inner dim on partitions)
  B: K^T [512, 2048] = wk^T @ ctxT ;  V [2048, 520] = ctxT^T @ wv
     (V natural orientation; per-head 65th column set to 1.0 so that the
      P'@V_ext matmul also produces the softmax denominators)
  C: per head pair (row-packed K=64 matmuls) and 256-wide query block:
       S^T [kv, nq] = K^T.T @ Q^T   (scores; no max-subtraction needed:
                                     |S| <= ~3 by construction)
       P'^T = exp(S^T)              (ACT, fused PSUM->SBUF)
       O'^T [65, nq] = V_ext.T @ P'^T   (row 64 = sum_j P' = denominators)
       O^T = O'^T[0:64] * broadcast(1/O'^T[64])
  D: out [2048, 1024] = O^T.T @ wo      (partial; host adds pair + bias)
"""
import sys

sys.path.insert(0, "/opt/trn_rl_repo")

import numpy as np

import concourse.bass as bass  # noqa: F401  (bass types used via tile/bacc)
import concourse.tile as tile
from concourse import bacc, mybir
from concourse import bass_utils

# Problem constants (hardcoded per harness contract).
B = 4
NQ = 2048
NKV = 2048
IN_DIM = 1024
CTX_DIM = 768
N_HEADS = 16
HEAD_DIM = 64
G = 512          # inner dim per core (8 heads)
HPC = 8          # heads per core
OUT_DIM = 1024
SCALE = HEAD_DIM ** -0.5

NQB = 256        # query block (f32r needs moving dim >= 256 for full rate)
NQBLKS = NQ // NQB            # 8
KVC = NKV // 128              # 16 kv chunks
VW = HEAD_DIM + 1             # 65: V columns per head incl. ones column
PACK_S = True                 # row-pack head pairs in the S matmul (K=64)

_CACHE = {}


def _build_program(pack_s=PACK_S):
    f32 = mybir.dt.float32
    f32r = mybir.dt.float32r
    EXP = mybir.ActivationFunctionType.Exp

    nc = bacc.Bacc("TRN2", target_bir_lowering=False, debug=False,
                   enable_asserts=False, num_devices=8)
    xT_d = nc.dram_tensor("xT", [IN_DIM, NQ], f32r, kind="ExternalInput").ap()
    ctxT_d = nc.dram_tensor("ctxT", [CTX_DIM, NKV], f32r, kind="ExternalInput").ap()
    wq_d = nc.dram_tensor("wq", [IN_DIM, G], f32r, kind="ExternalInput").ap()
    wk_d = nc.dram_tensor("wk", [CTX_DIM, G], f32r, kind="ExternalInput").ap()
    wv_d = nc.dram_tensor("wv", [CTX_DIM, G], f32r, kind="ExternalInput").ap()
    wo_d = nc.dram_tensor("wo", [G, OUT_DIM], f32r, kind="ExternalInput").ap()
    out_d = nc.dram_tensor("out", [NQ, OUT_DIM], f32, kind="ExternalOutput").ap()

    KQ = IN_DIM // 128   # 8 contraction chunks for Q proj
    KC = CTX_DIM // 128  # 6 contraction chunks for K/V proj
    MC = G // 128        # 4 inner chunks (head pairs)

    with tile.TileContext(nc, trace_sim=False) as tc:
        from contextlib import ExitStack
        with ExitStack() as ctx:
            # Persistent tensors (live across phases).
            pQT = ctx.enter_context(tc.tile_pool(name="qt", bufs=1))
            pKT = ctx.enter_context(tc.tile_pool(name="kt", bufs=1))
            pV = ctx.enter_context(tc.tile_pool(name="vv", bufs=1))
            QT = pQT.tile([128, MC * NQ], f32r)    # chunk m at free [m*NQ, (m+1)*NQ)
            KT = pKT.tile([128, MC * NKV], f32r)
            V = pV.tile([128, KVC * HPC * VW], f32r)  # chunk kvc at [kvc*520, +520)

            # --- Phase A: Q^T = wq^T @ xT ---------------------------------
            with tc.tile_pool(name="xt", bufs=1) as pxT, \
                 tc.tile_pool(name="wq", bufs=1) as pwq, \
                 tc.tile_pool(name="psA", bufs=4, space="PSUM") as psA:
                xT = pxT.tile([128, KQ * NQ], f32r)
                nc.sync.dma_start(
                    xT[:].rearrange("p (c n) -> p c n", c=KQ),
                    xT_d.rearrange("(c p) n -> p c n", p=128))
                wq = pwq.tile([128, KQ * G], f32r)
                nc.sync.dma_start(
                    wq[:].rearrange("p (c n) -> p c n", c=KQ),
                    wq_d.rearrange("(c p) n -> p c n", p=128))
                for m in range(MC):
                    for q in range(NQ // 512):
                        ps = psA.tile([128, 512], f32)
                        for k in range(KQ):
                            nc.tensor.matmul(
                                ps[:],
                                wq[:, k * G + m * 128:k * G + (m + 1) * 128],
                                xT[:, k * NQ + q * 512:k * NQ + (q + 1) * 512],
                                start=(k == 0), stop=(k == KQ - 1))
                        nc.vector.tensor_copy(
                            QT[:, m * NQ + q * 512:m * NQ + (q + 1) * 512], ps[:])

            # --- Phase B: K^T = wk^T @ ctxT ; V = ctxT^T @ wv -------------
            with tc.tile_pool(name="ct", bufs=1) as pcT, \
                 tc.tile_pool(name="wk", bufs=1) as pwk, \
                 tc.tile_pool(name="wv", bufs=1) as pwv, \
                 tc.tile_pool(name="psB", bufs=4, space="PSUM") as psB:
                ctxT = pcT.tile([128, KC * NKV], f32r)
                nc.sync.dma_start(
                    ctxT[:].rearrange("p (c n) -> p c n", c=KC),
                    ctxT_d.rearrange("(c p) n -> p c n", p=128))
                wk = pwk.tile([128, KC * G], f32r)
                nc.sync.dma_start(
                    wk[:].rearrange("p (c n) -> p c n", c=KC),
                    wk_d.rearrange("(c p) n -> p c n", p=128))
                wv = pwv.tile([128, KC * G], f32r)
                nc.sync.dma_start(
                    wv[:].rearrange("p (c n) -> p c n", c=KC),
                    wv_d.rearrange("(c p) n -> p c n", p=128))
                for m in range(MC):
                    for q in range(NKV // 512):
                        ps = psB.tile([128, 512], f32, tag="pskt")
                        for k in range(KC):
                            nc.tensor.matmul(
                                ps[:],
                                wk[:, k * G + m * 128:k * G + (m + 1) * 128],
                                ctxT[:, k * NKV + q * 512:k * NKV + (q + 1) * 512],
                                start=(k == 0), stop=(k == KC - 1))
                        nc.vector.tensor_copy(
                            KT[:, m * NKV + q * 512:m * NKV + (q + 1) * 512], ps[:])
                # ones columns for the denominator trick
                nc.gpsimd.memset(
                    V[:].bitcast(f32)
                    .rearrange("p (c h e) -> p c h e", c=KVC, e=VW)[:, :, :, 64:65],
                    1.0)
                for kvc in range(KVC):
                    ps = psB.tile([128, 512], f32, tag="psv")
                    for k in range(KC):
                        nc.tensor.matmul(
                            ps[:],
                            ctxT[:, k * NKV + kvc * 128:k * NKV + (kvc + 1) * 128],
                            wv[:, k * G:(k + 1) * G],
                            start=(k == 0), stop=(k == KC - 1))
                    nc.vector.tensor_copy(
                        V[:, kvc * HPC * VW:(kvc + 1) * HPC * VW]
                        .rearrange("p (h e) -> p h e", e=VW)[:, :, 0:64],
                        ps[:].rearrange("p (h e) -> p h e", e=64))

            # --- Phase C: attention ---------------------------------------
            # OT allocated here (not earlier) to keep phase A/B under the
            # SBUF cap; it persists through phase D via the outer ExitStack.
            pOT = ctx.enter_context(tc.tile_pool(name="ot", bufs=1))
            OT = pOT.tile([128, MC * NQ], f32r)
            with tc.tile_pool(name="pq", bufs=10) as pP, \
                 tc.tile_pool(name="den", bufs=4) as pDen, \
                 tc.tile_pool(name="psS", bufs=3, space="PSUM") as psS, \
                 tc.tile_pool(name="psO", bufs=2, space="PSUM") as psO:
                for m in range(MC):
                    for q in range(NQBLKS):
                        qo = q * NQB
                        quads = ([], [])  # P' quad tiles for h1, h2
                        for kv4 in range(KVC // 4):
                            s1 = psS.tile([128, 4 * NQB], f32, tag="s")
                            s2 = psS.tile([128, 4 * NQB], f32, tag="s")
                            for j in range(4):
                                kvc = kv4 * 4 + j
                                ko = m * NKV + kvc * 128
                                kw = dict(start=True, stop=True)
                                if pack_s:
                                    nc.tensor.matmul(
                                        s1[:, j * NQB:(j + 1) * NQB],
                                        KT[0:64, ko:ko + 128],
                                        QT[0:64, m * NQ + qo:m * NQ + qo + NQB],
                                        tile_position=(0, 0), **kw)
                                    nc.tensor.matmul(
                                        s2[:, j * NQB:(j + 1) * NQB],
                                        KT[64:128, ko:ko + 128],
                                        QT[64:128, m * NQ + qo:m * NQ + qo + NQB],
                                        tile_position=(64, 0), **kw)
                                else:
                                    nc.tensor.matmul(
                                        s1[:, j * NQB:(j + 1) * NQB],
                                        KT[0:64, ko:ko + 128],
                                        QT[0:64, m * NQ + qo:m * NQ + qo + NQB], **kw)
                                    nc.tensor.matmul(
                                        s2[:, j * NQB:(j + 1) * NQB],
                                        KT[64:128, ko:ko + 128],
                                        QT[64:128, m * NQ + qo:m * NQ + qo + NQB], **kw)
                            for hi, s in ((0, s1), (1, s2)):
                                pquad = pP.tile([128, 4 * NQB], f32r, tag="pq")
                                nc.scalar.activation(pquad[:], s[:], EXP)
                                quads[hi].append(pquad)
                        for hi in range(2):
                            h = 2 * m + hi
                            po = psO.tile([VW, NQB], f32)
                            for kvc in range(KVC):
                                nc.tensor.matmul(
                                    po[:],
                                    V[:, kvc * HPC * VW + h * VW:
                                       kvc * HPC * VW + (h + 1) * VW],
                                    quads[hi][kvc // 4][:, (kvc % 4) * NQB:
                                                        (kvc % 4 + 1) * NQB],
                                    start=(kvc == 0), stop=(kvc == KVC - 1))
                            d = pDen.tile([1, NQB], f32, tag="d")
                            nc.vector.reciprocal(d[:], po[64:65, :])
                            R = pDen.tile([64, NQB], f32, tag="r")
                            nc.gpsimd.partition_broadcast(R[:], d[:])
                            nc.vector.tensor_mul(
                                OT[hi * 64:(hi + 1) * 64, m * NQ + qo:m * NQ + qo + NQB],
                                po[0:64, :], R[:])

            # --- Phase D: out = O^T.T @ wo --------------------------------
            with tc.tile_pool(name="wo", bufs=1) as pwo, \
                 tc.tile_pool(name="outst", bufs=3) as pOut, \
                 tc.tile_pool(name="psD", bufs=3, space="PSUM") as psD:
                wo = pwo.tile([128, MC * OUT_DIM], f32r)
                nc.sync.dma_start(
                    wo[:].rearrange("p (c n) -> p c n", c=MC),
                    wo_d.rearrange("(c p) n -> p c n", p=128))
                for mq in range(NQ // 128):
                    for n2 in range(OUT_DIM // 512):
                        ps = psD.tile([128, 512], f32)
                        for c in range(MC):
                            nc.tensor.matmul(
                                ps[:],
                                OT[:, c * NQ + mq * 128:c * NQ + (mq + 1) * 128],
                                wo[:, c * OUT_DIM + n2 * 512:c * OUT_DIM + (n2 + 1) * 512],
                                start=(c == 0), stop=(c == MC - 1))
                        ob = pOut.tile([128, 512], f32)
                        nc.vector.tensor_copy(ob[:], ps[:])
                        nc.sync.dma_start(
                            out_d[mq * 128:(mq + 1) * 128, n2 * 512:(n2 + 1) * 512],
                            ob[:])

    nc.compile()
    return nc


def get_program(pack_s=PACK_S):
    key = ("prog", pack_s)
    if key not in _CACHE:
        _CACHE[key] = _build_program(pack_s)
    return _CACHE[key]


def make_in_maps(x, context, Wq, Wk, Wv, Wo):
    x = np.asarray(x, dtype=np.float32)
    context = np.asarray(context, dtype=np.float32)
    Wq = np.asarray(Wq, dtype=np.float32)
    Wk = np.asarray(Wk, dtype=np.float32)
    Wv = np.asarray(Wv, dtype=np.float32)
    Wo = np.asarray(Wo, dtype=np.float32)
    xT = [np.ascontiguousarray(x[b].T) for b in range(B)]
    ctxT = [np.ascontiguousarray(context[b].T) for b in range(B)]
    wq = [np.ascontiguousarray(Wq[:, g * G:(g + 1) * G]) * np.float32(SCALE)
          for g in range(2)]
    wk = [np.ascontiguousarray(Wk[:, g * G:(g + 1) * G]) for g in range(2)]
    wv = [np.ascontiguousarray(Wv[:, g * G:(g + 1) * G]) for g in range(2)]
    wo = [np.ascontiguousarray(Wo[g * G:(g + 1) * G, :]) for g in range(2)]
    in_maps = []
    for c in range(8):
        b, g = c // 2, c % 2
        in_maps.append({"xT": xT[b], "ctxT": ctxT[b], "wq": wq[g],
                        "wk": wk[g], "wv": wv[g], "wo": wo[g]})
    return in_maps


def run_device(nc, in_maps):
    return bass_utils.run_bass_kernel_spmd(nc, in_maps, core_ids=list(range(8)))


def kernel(x, context, Wq, Wk, Wv, Wo, bo):
    nc = get_program()
    in_maps = make_in_maps(x, context, Wq, Wk, Wv, Wo)
    res = run_device(nc, in_maps)
    bo = np.asarray(bo, dtype=np.float32)
    out = np.empty((B, NQ, OUT_DIM), dtype=np.float32)
    for b in range(B):
        out[b] = res.results[2 * b]["out"] + res.results[2 * b + 1]["out"] + bo
    return out


# revision 5
# speedup vs baseline: 4120.1668x; 4120.1668x over previous
"""Cross-attention Trainium2 Bass kernel.

Reference computation (per batch b):
  q = x @ Wq;  k = ctx @ Wk;  v = ctx @ Wv        (16 heads x 64 dim)
  sim = q k^T / 8;  attn = softmax(sim);  out = (attn v) @ Wo + bo

Sharding: 8 cores = 4 batches x 2 head-groups (8 heads each).
Each core computes a partial output [2048, 1024] (its 8 heads' contribution
through Wo); the host sums the two partials per batch and adds the bias.

Per-core data layout (host-prepared, all fp32 bits run as float32r on PE):
  xT   [1024, 2048]  = x[b].T               (contraction dim on partitions)
  ctxT [ 768, 2048]  = context[b].T
  wq   [1024,  512]  = Wq[:, g*512:+512] * 0.125   (attn scale folded in)
  wk   [ 768,  512]  = Wk[:, g*512:+512]
  wv   [ 768,  512]  = Wv[:, g*512:+512]
  wo   [ 512, 1024]  = Wo[g*512:+512, :]

Device pipeline:
  A: Q^T [512, 2048] = wq^T @ xT        (psum; inner dim on partitions)
  B: K^T [512, 2048] = wk^T @ ctxT ;  V [2048, 520] = ctxT^T @ wv
     (V natural orientation; per-head 65th column set to 1.0 so that the
      P'@V_ext matmul also produces the softmax denominators)
  C: per head pair (row-packed K=64 matmuls) and 256-wide query block:
       S^T [kv, nq] = K^T.T @ Q^T   (scores; no max-subtraction needed:
                                     |S| <= ~3 by construction)
       P'^T = exp(S^T)              (ACT, fused PSUM->SBUF)
       O'^T [65, nq] = V_ext.T @ P'^T   (row 64 = sum_j P' = denominators)
       O^T = O'^T[0:64] * broadcast(1/O'^T[64])
  D: out [2048, 1024] = O^T.T @ wo      (partial; host adds pair + bias)
"""
import sys

sys.path.insert(0, "/opt/trn_rl_repo")

import numpy as np

import concourse.bass as bass  # noqa: F401  (bass types used via tile/bacc)
import concourse.tile as tile
from concourse import bacc, mybir
from concourse import bass_utils

# Problem constants (hardcoded per harness contract).
B = 4
NQ = 2048
NKV = 2048
IN_DIM = 1024
CTX_DIM = 768
N_HEADS = 16
HEAD_DIM = 64
G = 512          # inner dim per core (8 heads)
HPC = 8          # heads per core
OUT_DIM = 1024
SCALE = HEAD_DIM ** -0.5

NQB = 256        # query block (f32r needs moving dim >= 256 for full rate)
NQBLKS = NQ // NQB            # 8
KVC = NKV // 128              # 16 kv chunks
VW = HEAD_DIM + 1             # 65: V columns per head incl. ones column
PACK_S = True                 # row-pack head pairs in the S matmul (K=64)

_CACHE = {}


def _build_program(pack_s=PACK_S, reps=1):
    f32 = mybir.dt.float32
    f32r = mybir.dt.float32r
    EXP = mybir.ActivationFunctionType.Exp

    nc = bacc.Bacc("TRN2", target_bir_lowering=False, debug=False,
                   enable_asserts=False, num_devices=8)
    xT_d = nc.dram_tensor("xT", [IN_DIM, NQ], f32r, kind="ExternalInput").ap()
    ctxT_d = nc.dram_tensor("ctxT", [CTX_DIM, NKV], f32r, kind="ExternalInput").ap()
    wq_d = nc.dram_tensor("wq", [IN_DIM, G], f32r, kind="ExternalInput").ap()
    wk_d = nc.dram_tensor("wk", [CTX_DIM, G], f32r, kind="ExternalInput").ap()
    wv_d = nc.dram_tensor("wv", [CTX_DIM, G], f32r, kind="ExternalInput").ap()
    wo_d = nc.dram_tensor("wo", [G, OUT_DIM], f32r, kind="ExternalInput").ap()
    out_d = nc.dram_tensor("out", [NQ, OUT_DIM], f32, kind="ExternalOutput").ap()

    KQ = IN_DIM // 128   # 8 contraction chunks for Q proj
    KC = CTX_DIM // 128  # 6 contraction chunks for K/V proj
    MC = G // 128        # 4 inner chunks (head pairs)

    from contextlib import ExitStack

    def _emit(tc):
        with ExitStack() as ctx:
            # Persistent tensors (live across phases).
            pQT = ctx.enter_context(tc.tile_pool(name="qt", bufs=1))
            pKT = ctx.enter_context(tc.tile_pool(name="kt", bufs=1))
            pV = ctx.enter_context(tc.tile_pool(name="vv", bufs=1))
            QT = pQT.tile([128, MC * NQ], f32r)    # chunk m at free [m*NQ, (m+1)*NQ)
            KT = pKT.tile([128, MC * NKV], f32r)
            V = pV.tile([128, KVC * HPC * VW], f32r)  # chunk kvc at [kvc*520, +520)

            # --- Phase A: Q^T = wq^T @ xT ---------------------------------
            with tc.tile_pool(name="xt", bufs=1) as pxT, \
                 tc.tile_pool(name="wq", bufs=1) as pwq, \
                 tc.tile_pool(name="psA", bufs=4, space="PSUM") as psA:
                xT = pxT.tile([128, KQ * NQ], f32r)
                nc.sync.dma_start(
                    xT[:].rearrange("p (c n) -> p c n", c=KQ),
                    xT_d.rearrange("(c p) n -> p c n", p=128))
                wq = pwq.tile([128, KQ * G], f32r)
                nc.sync.dma_start(
                    wq[:].rearrange("p (c n) -> p c n", c=KQ),
                    wq_d.rearrange("(c p) n -> p c n", p=128))
                for m in range(MC):
                    for q in range(NQ // 512):
                        ps = psA.tile([128, 512], f32)
                        for k in range(KQ):
                            nc.tensor.matmul(
                                ps[:],
                                wq[:, k * G + m * 128:k * G + (m + 1) * 128],
                                xT[:, k * NQ + q * 512:k * NQ + (q + 1) * 512],
                                start=(k == 0), stop=(k == KQ - 1))
                        nc.vector.tensor_copy(
                            QT[:, m * NQ + q * 512:m * NQ + (q + 1) * 512], ps[:])

            # --- Phase B: K^T = wk^T @ ctxT ; V = ctxT^T @ wv -------------
            with tc.tile_pool(name="ct", bufs=1) as pcT, \
                 tc.tile_pool(name="wk", bufs=1) as pwk, \
                 tc.tile_pool(name="wv", bufs=1) as pwv, \
                 tc.tile_pool(name="psB", bufs=4, space="PSUM") as psB:
                ctxT = pcT.tile([128, KC * NKV], f32r)
                nc.sync.dma_start(
                    ctxT[:].rearrange("p (c n) -> p c n", c=KC),
                    ctxT_d.rearrange("(c p) n -> p c n", p=128))
                wk = pwk.tile([128, KC * G], f32r)
                nc.sync.dma_start(
                    wk[:].rearrange("p (c n) -> p c n", c=KC),
                    wk_d.rearrange("(c p) n -> p c n", p=128))
                wv = pwv.tile([128, KC * G], f32r)
                nc.sync.dma_start(
                    wv[:].rearrange("p (c n) -> p c n", c=KC),
                    wv_d.rearrange("(c p) n -> p c n", p=128))
                for m in range(MC):
                    for q in range(NKV // 512):
                        ps = psB.tile([128, 512], f32, tag="pskt")
                        for k in range(KC):
                            nc.tensor.matmul(
                                ps[:],
                                wk[:, k * G + m * 128:k * G + (m + 1) * 128],
                                ctxT[:, k * NKV + q * 512:k * NKV + (q + 1) * 512],
                                start=(k == 0), stop=(k == KC - 1))
                        nc.vector.tensor_copy(
                            KT[:, m * NKV + q * 512:m * NKV + (q + 1) * 512], ps[:])
                # ones columns for the denominator trick
                nc.gpsimd.memset(
                    V[:].bitcast(f32)
                    .rearrange("p (c h e) -> p c h e", c=KVC, e=VW)[:, :, :, 64:65],
                    1.0)
                for kvc in range(KVC):
                    ps = psB.tile([128, 512], f32, tag="psv")
                    for k in range(KC):
                        nc.tensor.matmul(
                            ps[:],
                            ctxT[:, k * NKV + kvc * 128:k * NKV + (kvc + 1) * 128],
                            wv[:, k * G:(k + 1) * G],
                            start=(k == 0), stop=(k == KC - 1))
                    nc.vector.tensor_copy(
                        V[:, kvc * HPC * VW:(kvc + 1) * HPC * VW]
                        .rearrange("p (h e) -> p h e", e=VW)[:, :, 0:64],
                        ps[:].rearrange("p (h e) -> p h e", e=64))

            # --- Phase C: attention ---------------------------------------
            # OT allocated here (not earlier) to keep phase A/B under the
            # SBUF cap; it persists through phase D via the outer ExitStack.
            pOT = ctx.enter_context(tc.tile_pool(name="ot", bufs=1))
            OT = pOT.tile([128, MC * NQ], f32r)
            with tc.tile_pool(name="pq", bufs=10) as pP, \
                 tc.tile_pool(name="den", bufs=4) as pDen, \
                 tc.tile_pool(name="psS", bufs=3, space="PSUM") as psS, \
                 tc.tile_pool(name="psO", bufs=2, space="PSUM") as psO:
                for m in range(MC):
                    for q in range(NQBLKS):
                        qo = q * NQB
                        quads = ([], [])  # P' quad tiles for h1, h2
                        for kv4 in range(KVC // 4):
                            s1 = psS.tile([128, 4 * NQB], f32, tag="s")
                            s2 = psS.tile([128, 4 * NQB], f32, tag="s")
                            for j in range(4):
                                kvc = kv4 * 4 + j
                                ko = m * NKV + kvc * 128
                                kw = dict(start=True, stop=True)
                                if pack_s:
                                    nc.tensor.matmul(
                                        s1[:, j * NQB:(j + 1) * NQB],
                                        KT[0:64, ko:ko + 128],
                                        QT[0:64, m * NQ + qo:m * NQ + qo + NQB],
                                        tile_position=(0, 0), **kw)
                                    nc.tensor.matmul(
                                        s2[:, j * NQB:(j + 1) * NQB],
                                        KT[64:128, ko:ko + 128],
                                        QT[64:128, m * NQ + qo:m * NQ + qo + NQB],
                                        tile_position=(64, 0), **kw)
                                else:
                                    nc.tensor.matmul(
                                        s1[:, j * NQB:(j + 1) * NQB],
                                        KT[0:64, ko:ko + 128],
                                        QT[0:64, m * NQ + qo:m * NQ + qo + NQB], **kw)
                                    nc.tensor.matmul(
                                        s2[:, j * NQB:(j + 1) * NQB],
                                        KT[64:128, ko:ko + 128],
                                        QT[64:128, m * NQ + qo:m * NQ + qo + NQB], **kw)
                            for hi, s in ((0, s1), (1, s2)):
                                pquad = pP.tile([128, 4 * NQB], f32r, tag="pq")
                                nc.scalar.activation(pquad[:], s[:], EXP)
                                quads[hi].append(pquad)
                        for hi in range(2):
                            h = 2 * m + hi
                            po = psO.tile([VW, NQB], f32)
                            for kvc in range(KVC):
                                nc.tensor.matmul(
                                    po[:],
                                    V[:, kvc * HPC * VW + h * VW:
                                       kvc * HPC * VW + (h + 1) * VW],
                                    quads[hi][kvc // 4][:, (kvc % 4) * NQB:
                                                        (kvc % 4 + 1) * NQB],
                                    start=(kvc == 0), stop=(kvc == KVC - 1))
                            d = pDen.tile([1, NQB], f32, tag="d")
                            nc.vector.reciprocal(d[:], po[64:65, :])
                            R = pDen.tile([64, NQB], f32, tag="r")
                            nc.gpsimd.partition_broadcast(R[:], d[:])
                            nc.vector.tensor_mul(
                                OT[hi * 64:(hi + 1) * 64, m * NQ + qo:m * NQ + qo + NQB],
                                po[0:64, :], R[:])

            # --- Phase D: out = O^T.T @ wo --------------------------------
            with tc.tile_pool(name="wo", bufs=1) as pwo, \
                 tc.tile_pool(name="outst", bufs=3) as pOut, \
                 tc.tile_pool(name="psD", bufs=3, space="PSUM") as psD:
                wo = pwo.tile([128, MC * OUT_DIM], f32r)
                nc.sync.dma_start(
                    wo[:].rearrange("p (c n) -> p c n", c=MC),
                    wo_d.rearrange("(c p) n -> p c n", p=128))
                for mq in range(NQ // 128):
                    for n2 in range(OUT_DIM // 512):
                        ps = psD.tile([128, 512], f32)
                        for c in range(MC):
                            nc.tensor.matmul(
                                ps[:],
                                OT[:, c * NQ + mq * 128:c * NQ + (mq + 1) * 128],
                                wo[:, c * OUT_DIM + n2 * 512:c * OUT_DIM + (n2 + 1) * 512],
                                start=(c == 0), stop=(c == MC - 1))
                        ob = pOut.tile([128, 512], f32)
                        nc.vector.tensor_copy(ob[:], ps[:])
                        nc.sync.dma_start(
                            out_d[mq * 128:(mq + 1) * 128, n2 * 512:(n2 + 1) * 512],
                            ob[:])

    with tile.TileContext(nc, trace_sim=False) as tc:
        if reps == 1:
            _emit(tc)
        else:
            with tc.For_i(0, reps, 1):
                _emit(tc)

    nc.compile()
    return nc


def get_program(pack_s=PACK_S, reps=1):
    key = ("prog", pack_s, reps)
    if key not in _CACHE:
        _CACHE[key] = _build_program(pack_s, reps)
    return _CACHE[key]


def make_in_maps(x, context, Wq, Wk, Wv, Wo):
    x = np.asarray(x, dtype=np.float32)
    context = np.asarray(context, dtype=np.float32)
    Wq = np.asarray(Wq, dtype=np.float32)
    Wk = np.asarray(Wk, dtype=np.float32)
    Wv = np.asarray(Wv, dtype=np.float32)
    Wo = np.asarray(Wo, dtype=np.float32)
    xT = [np.ascontiguousarray(x[b].T) for b in range(B)]
    ctxT = [np.ascontiguousarray(context[b].T) for b in range(B)]
    wq = [np.ascontiguousarray(Wq[:, g * G:(g + 1) * G]) * np.float32(SCALE)
          for g in range(2)]
    wk = [np.ascontiguousarray(Wk[:, g * G:(g + 1) * G]) for g in range(2)]
    wv = [np.ascontiguousarray(Wv[:, g * G:(g + 1) * G]) for g in range(2)]
    wo = [np.ascontiguousarray(Wo[g * G:(g + 1) * G, :]) for g in range(2)]
    in_maps = []
    for c in range(8):
        b, g = c // 2, c % 2
        in_maps.append({"xT": xT[b], "ctxT": ctxT[b], "wq": wq[g],
                        "wk": wk[g], "wv": wv[g], "wo": wo[g]})
    return in_maps


def run_device(nc, in_maps):
    return bass_utils.run_bass_kernel_spmd(nc, in_maps, core_ids=list(range(8)))


def kernel(x, context, Wq, Wk, Wv, Wo, bo):
    nc = get_program()
    in_maps = make_in_maps(x, context, Wq, Wk, Wv, Wo)
    res = run_device(nc, in_maps)
    bo = np.asarray(bo, dtype=np.float32)
    out = np.empty((B, NQ, OUT_DIM), dtype=np.float32)
    for b in range(B):
        out[b] = res.results[2 * b]["out"] + res.results[2 * b + 1]["out"] + bo
    return out


# revision 7
# speedup vs baseline: 4322.8487x; 1.0492x over previous
"""Cross-attention Trainium2 Bass kernel.

Reference computation (per batch b):
  q = x @ Wq;  k = ctx @ Wk;  v = ctx @ Wv        (16 heads x 64 dim)
  sim = q k^T / 8;  attn = softmax(sim);  out = (attn v) @ Wo + bo

Sharding: 8 cores = 4 batches x 2 head-groups (8 heads each).
Each core computes a partial output [2048, 1024] (its 8 heads' contribution
through Wo); the host sums the two partials per batch and adds the bias.

Per-core data layout (host-prepared, all fp32 bits run as float32r on PE):
  xT   [1024, 2048]  = x[b].T               (contraction dim on partitions)
  ctxT [ 768, 2048]  = context[b].T
  wq   [1024,  512]  = Wq[:, g*512:+512] * 0.125   (attn scale folded in)
  wk   [ 768,  512]  = Wk[:, g*512:+512]
  wv   [ 768,  512]  = Wv[:, g*512:+512]
  wo   [ 512, 1024]  = Wo[g*512:+512, :]

Device pipeline:
  A: Q^T [512, 2048] = wq^T @ xT        (psum; inner dim on partitions)
  B: K^T [512, 2048] = wk^T @ ctxT ;  V [2048, 520] = ctxT^T @ wv
     (V natural orientation; per-head 65th column set to 1.0 so that the
      P'@V_ext matmul also produces the softmax denominators)
  C: per head pair (row-packed K=64 matmuls) and 256-wide query block:
       S^T [kv, nq] = K^T.T @ Q^T   (scores; no max-subtraction needed:
                                     |S| <= ~3 by construction)
       P'^T = exp(S^T)              (ACT, fused PSUM->SBUF)
       O'^T [65, nq] = V_ext.T @ P'^T   (row 64 = sum_j P' = denominators)
       O^T = O'^T[0:64] * broadcast(1/O'^T[64])
  D: out [2048, 1024] = O^T.T @ wo      (partial; host adds pair + bias)
"""
import sys

sys.path.insert(0, "/opt/trn_rl_repo")

import numpy as np

import concourse.bass as bass  # noqa: F401  (bass types used via tile/bacc)
import concourse.tile as tile
from concourse import bacc, mybir
from concourse import bass_utils

# Problem constants (hardcoded per harness contract).
B = 4
NQ = 2048
NKV = 2048
IN_DIM = 1024
CTX_DIM = 768
N_HEADS = 16
HEAD_DIM = 64
G = 512          # inner dim per core (8 heads)
HPC = 8          # heads per core
OUT_DIM = 1024
SCALE = HEAD_DIM ** -0.5

NQB = 256        # query block (f32r needs moving dim >= 256 for full rate)
NQBLKS = NQ // NQB            # 8
KVC = NKV // 128              # 16 kv chunks
VW = HEAD_DIM + 1             # 65: V columns per head incl. ones column
PACK_S = True                 # row-pack head pairs in the S matmul (K=64)

_CACHE = {}


def _build_program(pack_s=PACK_S, reps=1, mmdt="float32r"):
    f32 = mybir.dt.float32
    f32r = getattr(mybir.dt, mmdt)
    EXP = mybir.ActivationFunctionType.Exp

    nc = bacc.Bacc("TRN2", target_bir_lowering=False, debug=False,
                   enable_asserts=False, num_devices=8)
    xT_d = nc.dram_tensor("xT", [IN_DIM, NQ], f32r, kind="ExternalInput").ap()
    ctxT_d = nc.dram_tensor("ctxT", [CTX_DIM, NKV], f32r, kind="ExternalInput").ap()
    wq_d = nc.dram_tensor("wq", [IN_DIM, G], f32r, kind="ExternalInput").ap()
    wk_d = nc.dram_tensor("wk", [CTX_DIM, G], f32r, kind="ExternalInput").ap()
    wv_d = nc.dram_tensor("wv", [CTX_DIM, G], f32r, kind="ExternalInput").ap()
    wo_d = nc.dram_tensor("wo", [G, OUT_DIM], f32r, kind="ExternalInput").ap()
    out_d = nc.dram_tensor("out", [NQ, OUT_DIM], f32, kind="ExternalOutput").ap()

    KQ = IN_DIM // 128   # 8 contraction chunks for Q proj
    KC = CTX_DIM // 128  # 6 contraction chunks for K/V proj
    MC = G // 128        # 4 inner chunks (head pairs)

    from contextlib import ExitStack

    def _emit(tc):
        with ExitStack() as ctx:
            # Persistent tensors (live across phases).
            pQT = ctx.enter_context(tc.tile_pool(name="qt", bufs=1))
            pKT = ctx.enter_context(tc.tile_pool(name="kt", bufs=1))
            pV = ctx.enter_context(tc.tile_pool(name="vv", bufs=1))
            QT = pQT.tile([128, MC * NQ], f32r)    # chunk m at free [m*NQ, (m+1)*NQ)
            KT = pKT.tile([128, MC * NKV], f32r)
            V = pV.tile([128, KVC * HPC * VW], f32r)  # chunk kvc at [kvc*520, +520)

            # --- Phase A: Q^T = wq^T @ xT ---------------------------------
            with tc.tile_pool(name="xt", bufs=1) as pxT, \
                 tc.tile_pool(name="wq", bufs=1) as pwq, \
                 tc.tile_pool(name="psA", bufs=4, space="PSUM") as psA:
                xT = pxT.tile([128, KQ * NQ], f32r)
                nc.sync.dma_start(
                    xT[:].rearrange("p (c n) -> p c n", c=KQ),
                    xT_d.rearrange("(c p) n -> p c n", p=128))
                wq = pwq.tile([128, KQ * G], f32r)
                nc.sync.dma_start(
                    wq[:].rearrange("p (c n) -> p c n", c=KQ),
                    wq_d.rearrange("(c p) n -> p c n", p=128))
                for m in range(MC):
                    for q in range(NQ // 512):
                        ps = psA.tile([128, 512], f32)
                        for k in range(KQ):
                            nc.tensor.matmul(
                                ps[:],
                                wq[:, k * G + m * 128:k * G + (m + 1) * 128],
                                xT[:, k * NQ + q * 512:k * NQ + (q + 1) * 512],
                                start=(k == 0), stop=(k == KQ - 1))
                        nc.vector.tensor_copy(
                            QT[:, m * NQ + q * 512:m * NQ + (q + 1) * 512], ps[:])

            # --- Phase B: K^T = wk^T @ ctxT ; V = ctxT^T @ wv -------------
            with tc.tile_pool(name="ct", bufs=1) as pcT, \
                 tc.tile_pool(name="wk", bufs=1) as pwk, \
                 tc.tile_pool(name="wv", bufs=1) as pwv, \
                 tc.tile_pool(name="psB", bufs=4, space="PSUM") as psB:
                ctxT = pcT.tile([128, KC * NKV], f32r)
                nc.sync.dma_start(
                    ctxT[:].rearrange("p (c n) -> p c n", c=KC),
                    ctxT_d.rearrange("(c p) n -> p c n", p=128))
                wk = pwk.tile([128, KC * G], f32r)
                nc.sync.dma_start(
                    wk[:].rearrange("p (c n) -> p c n", c=KC),
                    wk_d.rearrange("(c p) n -> p c n", p=128))
                wv = pwv.tile([128, KC * G], f32r)
                nc.sync.dma_start(
                    wv[:].rearrange("p (c n) -> p c n", c=KC),
                    wv_d.rearrange("(c p) n -> p c n", p=128))
                for m in range(MC):
                    for q in range(NKV // 512):
                        ps = psB.tile([128, 512], f32, tag="pskt")
                        for k in range(KC):
                            nc.tensor.matmul(
                                ps[:],
                                wk[:, k * G + m * 128:k * G + (m + 1) * 128],
                                ctxT[:, k * NKV + q * 512:k * NKV + (q + 1) * 512],
                                start=(k == 0), stop=(k == KC - 1))
                        nc.vector.tensor_copy(
                            KT[:, m * NKV + q * 512:m * NKV + (q + 1) * 512], ps[:])
                # ones columns for the denominator trick (memset rejects
                # float32r, so write those bits through a float32 view)
                ones_view = V[:].bitcast(f32) if mmdt == "float32r" else V[:]
                nc.gpsimd.memset(
                    ones_view
                    .rearrange("p (c h e) -> p c h e", c=KVC, e=VW)[:, :, :, 64:65],
                    1.0)
                for kvc in range(KVC):
                    ps = psB.tile([128, 512], f32, tag="psv")
                    for k in range(KC):
                        nc.tensor.matmul(
                            ps[:],
                            ctxT[:, k * NKV + kvc * 128:k * NKV + (kvc + 1) * 128],
                            wv[:, k * G:(k + 1) * G],
                            start=(k == 0), stop=(k == KC - 1))
                    nc.vector.tensor_copy(
                        V[:, kvc * HPC * VW:(kvc + 1) * HPC * VW]
                        .rearrange("p (h e) -> p h e", e=VW)[:, :, 0:64],
                        ps[:].rearrange("p (h e) -> p h e", e=64))

            # --- Phase C: attention ---------------------------------------
            # OT allocated here (not earlier) to keep phase A/B under the
            # SBUF cap; it persists through phase D via the outer ExitStack.
            pOT = ctx.enter_context(tc.tile_pool(name="ot", bufs=1))
            OT = pOT.tile([128, MC * NQ], f32r)
            with tc.tile_pool(name="pq", bufs=10) as pP, \
                 tc.tile_pool(name="den", bufs=4) as pDen, \
                 tc.tile_pool(name="psS", bufs=3, space="PSUM") as psS, \
                 tc.tile_pool(name="psO", bufs=2, space="PSUM") as psO:
                for m in range(MC):
                    for q in range(NQBLKS):
                        qo = q * NQB
                        quads = ([], [])  # P' quad tiles for h1, h2
                        for kv4 in range(KVC // 4):
                            s1 = psS.tile([128, 4 * NQB], f32, tag="s")
                            s2 = psS.tile([128, 4 * NQB], f32, tag="s")
                            for j in range(4):
                                kvc = kv4 * 4 + j
                                ko = m * NKV + kvc * 128
                                kw = dict(start=True, stop=True)
                                if pack_s:
                                    nc.tensor.matmul(
                                        s1[:, j * NQB:(j + 1) * NQB],
                                        KT[0:64, ko:ko + 128],
                                        QT[0:64, m * NQ + qo:m * NQ + qo + NQB],
                                        tile_position=(0, 0), **kw)
                                    nc.tensor.matmul(
                                        s2[:, j * NQB:(j + 1) * NQB],
                                        KT[64:128, ko:ko + 128],
                                        QT[64:128, m * NQ + qo:m * NQ + qo + NQB],
                                        tile_position=(64, 0), **kw)
                                else:
                                    nc.tensor.matmul(
                                        s1[:, j * NQB:(j + 1) * NQB],
                                        KT[0:64, ko:ko + 128],
                                        QT[0:64, m * NQ + qo:m * NQ + qo + NQB], **kw)
                                    nc.tensor.matmul(
                                        s2[:, j * NQB:(j + 1) * NQB],
                                        KT[64:128, ko:ko + 128],
                                        QT[64:128, m * NQ + qo:m * NQ + qo + NQB], **kw)
                            for hi, s in ((0, s1), (1, s2)):
                                pquad = pP.tile([128, 4 * NQB], f32r, tag="pq")
                                nc.scalar.activation(pquad[:], s[:], EXP)
                                quads[hi].append(pquad)
                        for hi in range(2):
                            h = 2 * m + hi
                            po = psO.tile([VW, NQB], f32)
                            for kvc in range(KVC):
                                nc.tensor.matmul(
                                    po[:],
                                    V[:, kvc * HPC * VW + h * VW:
                                       kvc * HPC * VW + (h + 1) * VW],
                                    quads[hi][kvc // 4][:, (kvc % 4) * NQB:
                                                        (kvc % 4 + 1) * NQB],
                                    start=(kvc == 0), stop=(kvc == KVC - 1))
                            d = pDen.tile([1, NQB], f32, tag="d")
                            nc.vector.reciprocal(d[:], po[64:65, :])
                            R = pDen.tile([64, NQB], f32, tag="r")
                            nc.gpsimd.partition_broadcast(R[:], d[:])
                            nc.vector.tensor_mul(
                                OT[hi * 64:(hi + 1) * 64, m * NQ + qo:m * NQ + qo + NQB],
                                po[0:64, :], R[:])

            # --- Phase D: out = O^T.T @ wo --------------------------------
            with tc.tile_pool(name="wo", bufs=1) as pwo, \
                 tc.tile_pool(name="outst", bufs=3) as pOut, \
                 tc.tile_pool(name="psD", bufs=3, space="PSUM") as psD:
                wo = pwo.tile([128, MC * OUT_DIM], f32r)
                nc.sync.dma_start(
                    wo[:].rearrange("p (c n) -> p c n", c=MC),
                    wo_d.rearrange("(c p) n -> p c n", p=128))
                for mq in range(NQ // 128):
                    for n2 in range(OUT_DIM // 512):
                        ps = psD.tile([128, 512], f32)
                        for c in range(MC):
                            nc.tensor.matmul(
                                ps[:],
                                OT[:, c * NQ + mq * 128:c * NQ + (mq + 1) * 128],
                                wo[:, c * OUT_DIM + n2 * 512:c * OUT_DIM + (n2 + 1) * 512],
                                start=(c == 0), stop=(c == MC - 1))
                        ob = pOut.tile([128, 512], f32)
                        nc.vector.tensor_copy(ob[:], ps[:])
                        nc.sync.dma_start(
                            out_d[mq * 128:(mq + 1) * 128, n2 * 512:(n2 + 1) * 512],
                            ob[:])

    with tile.TileContext(nc, trace_sim=False) as tc:
        if reps == 1:
            _emit(tc)
        else:
            with tc.For_i(0, reps, 1):
                _emit(tc)

    nc.compile()
    return nc


def get_program(pack_s=PACK_S, reps=1, mmdt="float32r"):
    key = ("prog", pack_s, reps, mmdt)
    if key not in _CACHE:
        _CACHE[key] = _build_program(pack_s, reps, mmdt)
    return _CACHE[key]


def make_in_maps(x, context, Wq, Wk, Wv, Wo, mmdt="float32r"):
    import ml_dtypes
    hdt = np.float32 if mmdt == "float32r" else np.dtype(ml_dtypes.bfloat16)
    x = np.asarray(x, dtype=np.float32)
    context = np.asarray(context, dtype=np.float32)
    Wq = np.asarray(Wq, dtype=np.float32)
    Wk = np.asarray(Wk, dtype=np.float32)
    Wv = np.asarray(Wv, dtype=np.float32)
    Wo = np.asarray(Wo, dtype=np.float32)
    xT = [np.ascontiguousarray(x[b].T).astype(hdt) for b in range(B)]
    ctxT = [np.ascontiguousarray(context[b].T).astype(hdt) for b in range(B)]
    wq = [(np.ascontiguousarray(Wq[:, g * G:(g + 1) * G]) * np.float32(SCALE))
          .astype(hdt) for g in range(2)]
    wk = [np.ascontiguousarray(Wk[:, g * G:(g + 1) * G]).astype(hdt) for g in range(2)]
    wv = [np.ascontiguousarray(Wv[:, g * G:(g + 1) * G]).astype(hdt) for g in range(2)]
    wo = [np.ascontiguousarray(Wo[g * G:(g + 1) * G, :]).astype(hdt) for g in range(2)]
    in_maps = []
    for c in range(8):
        b, g = c // 2, c % 2
        in_maps.append({"xT": xT[b], "ctxT": ctxT[b], "wq": wq[g],
                        "wk": wk[g], "wv": wv[g], "wo": wo[g]})
    return in_maps


def run_device(nc, in_maps):
    return bass_utils.run_bass_kernel_spmd(nc, in_maps, core_ids=list(range(8)))


def kernel(x, context, Wq, Wk, Wv, Wo, bo, mmdt="float32r"):
    nc = get_program(mmdt=mmdt)
    in_maps = make_in_maps(x, context, Wq, Wk, Wv, Wo, mmdt=mmdt)
    res = run_device(nc, in_maps)
    bo = np.asarray(bo, dtype=np.float32)
    out = np.empty((B, NQ, OUT_DIM), dtype=np.float32)
    for b in range(B):
        out[b] = res.results[2 * b]["out"] + res.results[2 * b + 1]["out"] + bo
    return out
